# revision 64
# baseline (speedup 1.0000x reference)
"""Batched sparse-dense matmul (COO SpMM) on 8 Trainium2 NeuronCores.

Problem: y[b, r] = sum_k vals[k] * x[b, cols[k]] where rows[k] == r.
  x: [128, 16384] f32, vals/rows/cols: [524288], y: [128, 8192] f32.

Strategy: at 0.39% density with a full 128-wide batch, a dense matmul
y = x @ M^T beats any per-nonzero gather on this hardware (SWDGE
descriptor generation costs ~4-9ns per gathered element, and GPSIMD
scatter/gather ops run at ~1 elem/cycle-per-partition — both orders of
magnitude off the tensor engine).  So:
  - Host: densify M^T into W [C, R], shard W's output columns across
    the 8 cores (1024 rows each), and pre-tile both x^T and W for the
    SBUF partition layout.
  - Dtypes: W is cast to fp8e3 (e3m4, 4-bit mantissa) after scaling so
    max|W| lands at the e3m4 max (15.5); the inverse scale is folded
    into x, which stays fp16.  Measured rel error ~1.34e-2 (the e4m3
    variant fails the 2e-2 gate at 2.6e-2; fp16 W gives 3e-4 but
    doubles the dominant DMA stream).  W traffic halves vs fp16:
    16 MiB/core, leaving the PE moving-column stream (131072 cols
    @ 2.4 GHz = 55us) as the critical path.
  - Device (per core): keep x^T resident in SBUF as 128 [128c x 128b]
    fp16 chunks (the matmul's stationary operand); stream W from HBM
    split into row-halves, one per HWDGE ring (sync/scalar) so both
    rings carry identical load; x slices are injected into whichever
    ring is less loaded, in PE-need order; the first chunks load
    unsplit from a small head tensor so the critical first matmuls
    wait on one DMA+semaphore.  PSUM accumulates over the 128 c-chunks
    into y[128b x 1024r] fp32; DVE casts to bf16 and both rings store.
  - Latency trims (per trace): a burst of dummy matmuls on scratch SBUF
    ramps the PE p-state (0.65 -> 2.4 GHz needs ~3us busy) and delays
    the first real matmul to ~12.5us, by when the DMA pipeline (which
    also ramps, ~250 GB/s over its first 10us) has buffered enough W
    that the stream runs gap-free; deep W prefetch (16 tiles/ring)
    rides out HBM slow episodes; the final W group runs all psum0
    matmuls before psum1's so psum0's cast+store hide under the
    stream's last ~1.7us.
  - Host: concatenate the per-core row slices (upcast bf16 -> f32).

Known dead ends (measured): fp8e4 DoubleRow halves PE cycles but fails
the accuracy gate full-width (2.6e-2) and dies with a redacted runtime
INVALID_ARGUMENT when mixed in for the last 12 chunks at full kernel
scale (works in isolation); int8 is not a tensor-engine dtype; GPSIMD
scatter_add/ap_gather and SWDGE per-nnz gathers are 1-2 orders of
magnitude too slow for the 67M-MAC sparse form.
"""

import sys

sys.path.insert(0, "/opt/trn_rl_repo")

import ml_dtypes
import numpy as np

import concourse.bacc as bacc
import concourse.mybir as mybir
import concourse.tile as tile
from concourse.bass_utils import run_bass_kernel_spmd

B = 128        # batch
R = 8192       # rows of sparse matrix / output features
C = 16384      # cols of sparse matrix / input features
NCORES = 8
RC = R // NCORES       # rows (output features) per core
NCH = C // 128         # contraction chunks of 128
NT = RC // 512         # 512-wide PSUM column tiles per core (= 2)

E3M4_MAX = 15.5        # largest finite float8_e3m4

# W DMA tile sizes in c-chunks: a small leading tile cuts time-to-first-
# matmul, but too many small tiles serialize on HWDGE descriptor-gen
# (~0.63us per dma_start), starving the PE early.  The first N_HEAD chunks
# load unsplit (full 1024 rows) from a separate head tensor so the critical
# first matmuls wait on a single DMA+semaphore per group.
W_HEAD_GROUPS = [2, 2]
N_HEAD = sum(W_HEAD_GROUPS)
W_GROUPS = [4] + [8] * 15     # split lo/hi across the two rings
# x^T DMA slice sizes in c-chunks, same idea
X_SLICES = [4, 12] + [8] * 14
# Dummy matmuls ramp the PE p-state AND delay the first real matmul until
# the DMA pipeline has built a W cushion (see module docstring).
N_WARMUP = 24


def _starts(sizes, base=0, total=NCH):
    out, acc = [], base
    for s in sizes:
        out.append(acc)
        acc += s
    assert acc == total
    return out


W_HEAD_START = _starts(W_HEAD_GROUPS, 0, N_HEAD)
W_START = _starts(W_GROUPS, N_HEAD)
X_START = _starts(X_SLICES)


def _schedule():
    """DMA issue schedule: events in PE-need order, greedily assigned to the
    ring with fewer cumulative bytes.  Head W groups load unsplit; later W
    groups are split into lo/hi row halves (one per event) so both rings
    carry the same W load."""
    ev = []  # (need_chunk, prio, kind, idx, bytes)
    for s, st in enumerate(X_START):
        ev.append((st, 0, "x", s, X_SLICES[s] * B * 2 * 128))
    for g, st in enumerate(W_HEAD_START):
        ev.append((st, 1, "wh", g, W_HEAD_GROUPS[g] * RC * 128))
    for g, st in enumerate(W_START):
        ev.append((st, 1, "wlo", g, W_GROUPS[g] * 512 * 128))
        ev.append((st, 2, "whi", g, W_GROUPS[g] * 512 * 128))
    ev.sort()
    load = [0, 0]
    out = []  # (kind, idx, ring)
    for _, _, kind, idx, nb in ev:
        r = 0 if load[0] <= load[1] else 1
        load[r] += nb
        out.append((kind, idx, r))
    return out


def _densify_tiled(vals, rows, cols):
    """w_t[p, ch, r] = sum of vals at (row=r, col=ch*128+p): dense M^T
    pre-tiled for the SBUF partition layout, [128, NCH, R] f32."""
    w_t = np.zeros((128, NCH, R), dtype=np.float32)
    np.add.at(w_t, (cols % 128, cols // 128, rows), vals)
    return w_t


def _build_nc():
    nc = bacc.Bacc("TRN2", target_bir_lowering=False, debug=False)
    # x^T pre-tiled on host: xt[p, ch, b] = x[b, ch*128+p] / s
    xt_d = nc.dram_tensor(
        "xt", [128, NCH * B], mybir.dt.float16, kind="ExternalInput"
    )
    # W pre-tiled on host.  whead: first N_HEAD chunks, full row width.
    # w{lo,hi}[p, ch, r] = s * W[ch*128+p, core_rows[half*512 + r]]
    whead_d = nc.dram_tensor(
        "whead", [128, N_HEAD, RC], mybir.dt.float8e3, kind="ExternalInput"
    )
    wlo_d = nc.dram_tensor(
        "wlo", [128, NCH, 512], mybir.dt.float8e3, kind="ExternalInput"
    )
    whi_d = nc.dram_tensor(
        "whi", [128, NCH, 512], mybir.dt.float8e3, kind="ExternalInput"
    )
    y_d = nc.dram_tensor(
        "y", [128, RC], mybir.dt.bfloat16, kind="ExternalOutput"
    )
    w_d = [wlo_d, whi_d]
    rings = [nc.sync, nc.scalar]

    with tile.TileContext(nc) as tc:
        with (
            tc.tile_pool(name="xsb", bufs=1) as xpool,
            tc.tile_pool(name="wlosb", bufs=16) as wlopool,
            tc.tile_pool(name="whisb", bufs=16) as whipool,
            tc.tile_pool(name="ysb", bufs=1) as ypool,
            tc.tile_pool(name="ps", bufs=NT, space="PSUM") as ppool,
            tc.tile_pool(name="pwarm", bufs=1, space="PSUM") as wppool,
        ):
            # --- PE warmup: ramp the p-state while the first DMAs land ---
            # (the y output tile doubles as zeroed bf16 scratch; the WAR
            # dependency against the final casts is satisfied ~40us early)
            y_t = ypool.tile([128, RC], mybir.dt.bfloat16)
            nc.gpsimd.memset(y_t[:, :512], 0.0)
            wpsum = wppool.tile([128, 256], mybir.dt.float32)
            for _ in range(N_WARMUP):
                nc.tensor.matmul(
                    wpsum[:], y_t[:, :128], y_t[:, :256], start=True, stop=True
                )

            x_t = xpool.tile([128, NCH, B], mybir.dt.float16)
            wpools = [wlopool, whipool]
            w_tiles = [[], []]
            wh_tiles = []
            # issue every DMA in need order; tile-pool reuse semaphores pace
            # the W stream ~16 groups ahead of the matmuls
            for kind, idx, ring in _schedule():
                if kind == "x":
                    st, ln = X_START[idx], X_SLICES[idx]
                    rings[ring].dma_start(
                        out=x_t[:, st:st + ln, :],
                        in_=xt_d[:, st * B:(st + ln) * B],
                    )
                elif kind == "wh":
                    st, ln = W_HEAD_START[idx], W_HEAD_GROUPS[idx]
                    w_t = wlopool.tile([128, ln, RC], mybir.dt.float8e3)
                    wh_tiles.append(w_t)
                    rings[ring].dma_start(
                        out=w_t[:], in_=whead_d[:, st:st + ln, :]
                    )
                else:
                    t = 0 if kind == "wlo" else 1
                    st, ln = W_START[idx], W_GROUPS[idx]
                    w_t = wpools[t].tile([128, ln, 512], mybir.dt.float8e3)
                    w_tiles[t].append(w_t)
                    rings[ring].dma_start(
                        out=w_t[:], in_=w_d[t][:, st:st + ln, :]
                    )
            psums = [
                ppool.tile(
                    [128, 512], mybir.dt.float32, name=f"psum{t}", tag=f"psum{t}"
                )
                for t in range(NT)
            ]
            for g, (wst, wln) in enumerate(zip(W_HEAD_START, W_HEAD_GROUPS)):
                for i in range(wln):
                    ch = wst + i
                    for t in range(NT):
                        nc.tensor.matmul(
                            psums[t][:],
                            x_t[:, ch, :],
                            wh_tiles[g][:, i, t * 512:(t + 1) * 512],
                            start=(ch == 0),
                            stop=False,
                        )
            last_g = len(W_GROUPS) - 1
            for g, (wst, wln) in enumerate(zip(W_START, W_GROUPS)):
                if g < last_g:
                    order = [(i, t) for i in range(wln) for t in range(NT)]
                else:
                    # final group: finish psum0 first so its cast+store
                    # overlap the remaining psum1 matmuls
                    order = [(i, t) for t in range(NT) for i in range(wln)]
                for i, t in order:
                    ch = wst + i
                    nc.tensor.matmul(
                        psums[t][:],
                        x_t[:, ch, :],
                        w_tiles[t][g][:, i, :],
                        start=False,
                        stop=(ch == NCH - 1),
                    )
            # tail: cast each 512-row half to bf16 via DVE (halves the copy
            # and store; adds ~0.2% rounding, negligible next to the 1.3%
            # W-quantization error), store on its own ring.  The last half
            # (psum1, the final stop) casts in two 256-col pieces so its
            # first store overlaps the second cast.
            nc.vector.tensor_copy(out=y_t[:, 0:512], in_=psums[0][:])
            rings[0].dma_start(out=y_d[:, 0:512], in_=y_t[:, 0:512])
            for h in range(2):
                lo, hi = 512 + h * 256, 512 + (h + 1) * 256
                nc.vector.tensor_copy(
                    out=y_t[:, lo:hi], in_=psums[1][:, h * 256:(h + 1) * 256]
                )
                rings[1].dma_start(out=y_d[:, lo:hi], in_=y_t[:, lo:hi])
    nc.compile()
    return nc


_CACHE = {}
_TRACE = False  # set by bench harness to capture an NTFF profile


def _get_nc():
    if "nc" not in _CACHE:
        _CACHE["nc"] = _build_nc()
    return _CACHE["nc"]


def kernel(x_batched, M_vals, M_row_idx, M_col_idx, _want_results=False, **_):
    x = np.asarray(x_batched, dtype=np.float32)
    vals = np.asarray(M_vals, dtype=np.float32)
    rows = np.asarray(M_row_idx, dtype=np.int64)
    cols = np.asarray(M_col_idx, dtype=np.int64)

    w_t = _densify_tiled(vals, rows, cols)               # [128, NCH, R] f32
    s = E3M4_MAX / float(np.abs(w_t).max()) * 0.999
    w_t8 = (w_t * s).astype(ml_dtypes.float8_e3m4)
    xt = np.ascontiguousarray(
        (x.T / s).reshape(NCH, 128, B).transpose(1, 0, 2).reshape(128, NCH * B)
    ).astype(np.float16)

    nc = _get_nc()
    in_maps = [
        {
            "xt": xt,
            "whead": np.ascontiguousarray(
                w_t8[:, :N_HEAD, m * RC:(m + 1) * RC]
            ),
            "wlo": np.ascontiguousarray(
                w_t8[:, :, m * RC:m * RC + 512]
            ),
            "whi": np.ascontiguousarray(
                w_t8[:, :, m * RC + 512:(m + 1) * RC]
            ),
        }
        for m in range(NCORES)
    ]
    res = run_bass_kernel_spmd(
        nc, in_maps, core_ids=list(range(NCORES)), trace=_TRACE
    )

    y = np.empty((B, R), dtype=np.float32)
    for m in range(NCORES):
        y[:, m * RC:(m + 1) * RC] = np.asarray(
            res.results[m]["y"]
        ).astype(np.float32)
    if _want_results:
        return y, res
    return y


# revision 67
# speedup vs baseline: 1.0093x; 1.0093x over previous
"""Batched sparse-dense matmul (COO SpMM) on 8 Trainium2 NeuronCores.

Problem: y[b, r] = sum_k vals[k] * x[b, cols[k]] where rows[k] == r.
  x: [128, 16384] f32, vals/rows/cols: [524288], y: [128, 8192] f32.

Strategy: at 0.39% density with a full 128-wide batch, a dense matmul
y = x @ M^T beats any per-nonzero gather on this hardware (SWDGE
descriptor generation costs ~4-9ns per gathered element, and GPSIMD
scatter/gather ops run at ~1 elem/cycle-per-partition — both orders of
magnitude off the tensor engine).  So:
  - Host: densify M^T into W [C, R], shard W's output columns across
    the 8 cores (1024 rows each), and pre-tile both x^T and W for the
    SBUF partition layout.
  - Dtypes: W is cast to fp8e3 (e3m4, 4-bit mantissa) after scaling so
    max|W| lands at the e3m4 max (15.5); the inverse scale is folded
    into x, which stays fp16.  Measured rel error ~1.34e-2 (the e4m3
    variant fails the 2e-2 gate at 2.6e-2; fp16 W gives 3e-4 but
    doubles the dominant DMA stream).  W traffic halves vs fp16:
    16 MiB/core, leaving the PE moving-column stream (131072 cols
    @ 2.4 GHz = 55us) as the critical path.
  - Device (per core): keep x^T resident in SBUF as 128 [128c x 128b]
    fp16 chunks (the matmul's stationary operand); stream W from HBM
    split into row-halves, one per HWDGE ring (sync/scalar) so both
    rings carry identical load; x slices are injected into whichever
    ring is less loaded, in PE-need order; the first chunks load
    unsplit from a small head tensor so the critical first matmuls
    wait on one DMA+semaphore.  PSUM accumulates over the 128 c-chunks
    into y[128b x 1024r] fp32; DVE casts to bf16 and both rings store.
  - Latency trims (per trace): a burst of dummy matmuls on scratch SBUF
    ramps the PE p-state (0.65 -> 2.4 GHz needs ~3us busy) and delays
    the first real matmul to ~12.5us, by when the DMA pipeline (which
    also ramps, ~250 GB/s over its first 10us) has buffered enough W
    that the stream runs gap-free; deep W prefetch (16 tiles/ring)
    rides out HBM slow episodes; the final W group runs all psum0
    matmuls before psum1's so psum0's cast+store hide under the
    stream's last ~1.7us.
  - Host: concatenate the per-core row slices (upcast bf16 -> f32).

Known dead ends (measured): fp8e4 DoubleRow halves PE cycles but fails
the accuracy gate full-width (2.6e-2) and dies with a redacted runtime
INVALID_ARGUMENT when mixed in for the last 12 chunks at full kernel
scale (works in isolation); int8 is not a tensor-engine dtype; GPSIMD
scatter_add/ap_gather and SWDGE per-nnz gathers are 1-2 orders of
magnitude too slow for the 67M-MAC sparse form.
"""

import sys

sys.path.insert(0, "/opt/trn_rl_repo")

import ml_dtypes
import numpy as np

import concourse.bacc as bacc
import concourse.mybir as mybir
import concourse.tile as tile
from concourse.bass_utils import run_bass_kernel_spmd

B = 128        # batch
R = 8192       # rows of sparse matrix / output features
C = 16384      # cols of sparse matrix / input features
NCORES = 8
RC = R // NCORES       # rows (output features) per core
NCH = C // 128         # contraction chunks of 128
NT = RC // 512         # 512-wide PSUM column tiles per core (= 2)

E3M4_MAX = 15.5        # largest finite float8_e3m4

# W DMA tile sizes in c-chunks: a small leading tile cuts time-to-first-
# matmul, but too many small tiles serialize on HWDGE descriptor-gen
# (~0.63us per dma_start), starving the PE early.  The first N_HEAD chunks
# load unsplit (full 1024 rows) from a separate head tensor so the critical
# first matmuls wait on a single DMA+semaphore per group.
W_HEAD_GROUPS = [2, 2]
N_HEAD = sum(W_HEAD_GROUPS)
W_GROUPS = [4] + [8] * 15     # split lo/hi across the two rings
# x^T DMA slice sizes in c-chunks, same idea
X_SLICES = [4, 12] + [8] * 14
# Dummy matmuls ramp the PE p-state AND delay the first real matmul until
# the DMA pipeline has built a W cushion (see module docstring).
N_WARMUP = 24


def _starts(sizes, base=0, total=NCH):
    out, acc = [], base
    for s in sizes:
        out.append(acc)
        acc += s
    assert acc == total
    return out


W_HEAD_START = _starts(W_HEAD_GROUPS, 0, N_HEAD)
W_START = _starts(W_GROUPS, N_HEAD)
X_START = _starts(X_SLICES)


def _schedule():
    """DMA issue schedule: events in PE-need order, greedily assigned to the
    ring with fewer cumulative bytes.  Head W groups load unsplit; later W
    groups are split into lo/hi row halves (one per event) so both rings
    carry the same W load."""
    ev = []  # (need_chunk, prio, kind, idx, bytes)
    for s, st in enumerate(X_START):
        ev.append((st, 0, "x", s, X_SLICES[s] * B * 2 * 128))
    for g, st in enumerate(W_HEAD_START):
        ev.append((st, 1, "wh", g, W_HEAD_GROUPS[g] * RC * 128))
    for g, st in enumerate(W_START):
        ev.append((st, 1, "wlo", g, W_GROUPS[g] * 512 * 128))
        ev.append((st, 2, "whi", g, W_GROUPS[g] * 512 * 128))
    ev.sort()
    load = [0, 0]
    out = []  # (kind, idx, ring)
    for _, _, kind, idx, nb in ev:
        r = 0 if load[0] <= load[1] else 1
        load[r] += nb
        out.append((kind, idx, r))
    return out


def _densify_tiled(vals, rows, cols):
    """w_t[p, ch, r] = sum of vals at (row=r, col=ch*128+p): dense M^T
    pre-tiled for the SBUF partition layout, [128, NCH, R] f32."""
    w_t = np.zeros((128, NCH, R), dtype=np.float32)
    np.add.at(w_t, (cols % 128, cols // 128, rows), vals)
    return w_t


def _build_nc():
    nc = bacc.Bacc("TRN2", target_bir_lowering=False, debug=False)
    # x^T pre-tiled on host: xt[p, ch, b] = x[b, ch*128+p] / s
    xt_d = nc.dram_tensor(
        "xt", [128, NCH * B], mybir.dt.float16, kind="ExternalInput"
    )
    # W pre-tiled on host.  whead: first N_HEAD chunks, full row width.
    # w{lo,hi}[p, ch, r] = s * W[ch*128+p, core_rows[half*512 + r]]
    whead_d = nc.dram_tensor(
        "whead", [128, N_HEAD, RC], mybir.dt.float8e3, kind="ExternalInput"
    )
    wlo_d = nc.dram_tensor(
        "wlo", [128, NCH, 512], mybir.dt.float8e3, kind="ExternalInput"
    )
    whi_d = nc.dram_tensor(
        "whi", [128, NCH, 512], mybir.dt.float8e3, kind="ExternalInput"
    )
    y_d = nc.dram_tensor(
        "y", [128, RC], mybir.dt.bfloat16, kind="ExternalOutput"
    )
    w_d = [wlo_d, whi_d]
    rings = [nc.sync, nc.scalar]

    with tile.TileContext(nc) as tc:
        with (
            tc.tile_pool(name="xsb", bufs=1) as xpool,
            tc.tile_pool(name="whead", bufs=len(W_HEAD_GROUPS)) as whpool,
            tc.tile_pool(name="wlosb", bufs=16) as wlopool,
            tc.tile_pool(name="whisb", bufs=16) as whipool,
            tc.tile_pool(name="warm", bufs=1) as mpool,
            tc.tile_pool(name="ysb", bufs=1) as ypool,
            tc.tile_pool(name="ps", bufs=NT, space="PSUM") as ppool,
            tc.tile_pool(name="pwarm", bufs=1, space="PSUM") as wppool,
        ):
            # --- PE warmup: ramp the p-state while the first DMAs land ---
            warm = mpool.tile([128, 512], mybir.dt.float16)
            nc.gpsimd.memset(warm[:], 0.0)
            wpsum = wppool.tile([128, 256], mybir.dt.float32)
            for _ in range(N_WARMUP):
                nc.tensor.matmul(
                    wpsum[:], warm[:, :128], warm[:, :256], start=True, stop=True
                )

            x_t = xpool.tile([128, NCH, B], mybir.dt.float16)
            wpools = [wlopool, whipool]
            w_tiles = [[], []]
            wh_tiles = []
            # issue every DMA in need order; tile-pool reuse semaphores pace
            # the W stream ~16 groups ahead of the matmuls
            for kind, idx, ring in _schedule():
                if kind == "x":
                    st, ln = X_START[idx], X_SLICES[idx]
                    rings[ring].dma_start(
                        out=x_t[:, st:st + ln, :],
                        in_=xt_d[:, st * B:(st + ln) * B],
                    )
                elif kind == "wh":
                    st, ln = W_HEAD_START[idx], W_HEAD_GROUPS[idx]
                    w_t = whpool.tile([128, ln, RC], mybir.dt.float8e3)
                    wh_tiles.append(w_t)
                    rings[ring].dma_start(
                        out=w_t[:], in_=whead_d[:, st:st + ln, :]
                    )
                else:
                    t = 0 if kind == "wlo" else 1
                    st, ln = W_START[idx], W_GROUPS[idx]
                    w_t = wpools[t].tile([128, ln, 512], mybir.dt.float8e3)
                    w_tiles[t].append(w_t)
                    rings[ring].dma_start(
                        out=w_t[:], in_=w_d[t][:, st:st + ln, :]
                    )
            psums = [
                ppool.tile(
                    [128, 512], mybir.dt.float32, name=f"psum{t}", tag=f"psum{t}"
                )
                for t in range(NT)
            ]
            for g, (wst, wln) in enumerate(zip(W_HEAD_START, W_HEAD_GROUPS)):
                for i in range(wln):
                    ch = wst + i
                    for t in range(NT):
                        nc.tensor.matmul(
                            psums[t][:],
                            x_t[:, ch, :],
                            wh_tiles[g][:, i, t * 512:(t + 1) * 512],
                            start=(ch == 0),
                            stop=False,
                        )
            last_g = len(W_GROUPS) - 1
            for g, (wst, wln) in enumerate(zip(W_START, W_GROUPS)):
                if g < last_g:
                    order = [(i, t) for i in range(wln) for t in range(NT)]
                else:
                    # final group: finish psum0 first so its cast+store
                    # overlap the remaining psum1 matmuls
                    order = [(i, t) for t in range(NT) for i in range(wln)]
                for i, t in order:
                    ch = wst + i
                    nc.tensor.matmul(
                        psums[t][:],
                        x_t[:, ch, :],
                        w_tiles[t][g][:, i, :],
                        start=False,
                        stop=(ch == NCH - 1),
                    )
            # tail: cast each 512-row half to bf16 via DVE (halves the copy
            # and store; adds ~0.2% rounding, negligible next to the 1.3%
            # W-quantization error), store on its own ring.  The last half
            # (psum1, the final stop) casts in two 256-col pieces so its
            # first store overlaps the second cast.
            y_t = ypool.tile([128, RC], mybir.dt.bfloat16)
            nc.vector.tensor_copy(out=y_t[:, 0:512], in_=psums[0][:])
            rings[0].dma_start(out=y_d[:, 0:512], in_=y_t[:, 0:512])
            for h in range(2):
                lo, hi = 512 + h * 256, 512 + (h + 1) * 256
                nc.vector.tensor_copy(
                    out=y_t[:, lo:hi], in_=psums[1][:, h * 256:(h + 1) * 256]
                )
                rings[1].dma_start(out=y_d[:, lo:hi], in_=y_t[:, lo:hi])
    nc.compile()
    return nc


_CACHE = {}
_TRACE = False  # set by bench harness to capture an NTFF profile


def _get_nc():
    if "nc" not in _CACHE:
        _CACHE["nc"] = _build_nc()
    return _CACHE["nc"]


def kernel(x_batched, M_vals, M_row_idx, M_col_idx, _want_results=False, **_):
    x = np.asarray(x_batched, dtype=np.float32)
    vals = np.asarray(M_vals, dtype=np.float32)
    rows = np.asarray(M_row_idx, dtype=np.int64)
    cols = np.asarray(M_col_idx, dtype=np.int64)

    w_t = _densify_tiled(vals, rows, cols)               # [128, NCH, R] f32
    s = E3M4_MAX / float(np.abs(w_t).max()) * 0.999
    w_t8 = (w_t * s).astype(ml_dtypes.float8_e3m4)
    xt = np.ascontiguousarray(
        (x.T / s).reshape(NCH, 128, B).transpose(1, 0, 2).reshape(128, NCH * B)
    ).astype(np.float16)

    nc = _get_nc()
    in_maps = [
        {
            "xt": xt,
            "whead": np.ascontiguousarray(
                w_t8[:, :N_HEAD, m * RC:(m + 1) * RC]
            ),
            "wlo": np.ascontiguousarray(
                w_t8[:, :, m * RC:m * RC + 512]
            ),
            "whi": np.ascontiguousarray(
                w_t8[:, :, m * RC + 512:(m + 1) * RC]
            ),
        }
        for m in range(NCORES)
    ]
    res = run_bass_kernel_spmd(
        nc, in_maps, core_ids=list(range(NCORES)), trace=_TRACE
    )

    y = np.empty((B, R), dtype=np.float32)
    for m in range(NCORES):
        y[:, m * RC:(m + 1) * RC] = np.asarray(
            res.results[m]["y"]
        ).astype(np.float32)
    if _want_results:
        return y, res
    return y


# revision 69
# speedup vs baseline: 1.0140x; 1.0046x over previous
"""Batched sparse-dense matmul (COO SpMM) on 8 Trainium2 NeuronCores.

Problem: y[b, r] = sum_k vals[k] * x[b, cols[k]] where rows[k] == r.
  x: [128, 16384] f32, vals/rows/cols: [524288], y: [128, 8192] f32.

Strategy: at 0.39% density with a full 128-wide batch, a dense matmul
y = x @ M^T beats any per-nonzero gather on this hardware (SWDGE
descriptor generation costs ~4-9ns per gathered element, and GPSIMD
scatter/gather ops run at ~1 elem/cycle-per-partition — both orders of
magnitude off the tensor engine).  So:
  - Host: densify M^T into W [C, R], shard W's output columns across
    the 8 cores (1024 rows each), and pre-tile both x^T and W for the
    SBUF partition layout.
  - Dtypes: W is cast to fp8e3 (e3m4, 4-bit mantissa) after scaling so
    max|W| lands at the e3m4 max (15.5); the inverse scale is folded
    into x, which stays fp16.  Measured rel error ~1.34e-2 (the e4m3
    variant fails the 2e-2 gate at 2.6e-2; fp16 W gives 3e-4 but
    doubles the dominant DMA stream).  W traffic halves vs fp16:
    16 MiB/core, leaving the PE moving-column stream (131072 cols
    @ 2.4 GHz = 55us) as the critical path.
  - Device (per core): keep x^T resident in SBUF as 128 [128c x 128b]
    fp16 chunks (the matmul's stationary operand); stream W from HBM
    split into row-halves, one per HWDGE ring (sync/scalar) so both
    rings carry identical load; x slices are injected into whichever
    ring is less loaded, in PE-need order; the first chunks load
    unsplit from a small head tensor so the critical first matmuls
    wait on one DMA+semaphore.  PSUM accumulates over the 128 c-chunks
    into y[128b x 1024r] fp32; DVE casts to bf16 and both rings store.
  - Latency trims (per trace): a burst of dummy matmuls on scratch SBUF
    ramps the PE p-state (0.65 -> 2.4 GHz needs ~3us busy) and delays
    the first real matmul to ~12.5us, by when the DMA pipeline (which
    also ramps, ~250 GB/s over its first 10us) has buffered enough W
    that the stream runs gap-free; deep W prefetch (16 tiles/ring)
    rides out HBM slow episodes; the final W group runs all psum0
    matmuls before psum1's so psum0's cast+store hide under the
    stream's last ~1.7us.
  - Host: concatenate the per-core row slices (upcast bf16 -> f32).

Known dead ends (measured): fp8e4 DoubleRow halves PE cycles but fails
the accuracy gate full-width (2.6e-2) and dies with a redacted runtime
INVALID_ARGUMENT when mixed in for the last 12 chunks at full kernel
scale (works in isolation); int8 is not a tensor-engine dtype; GPSIMD
scatter_add/ap_gather and SWDGE per-nnz gathers are 1-2 orders of
magnitude too slow for the 67M-MAC sparse form.
"""

import sys

sys.path.insert(0, "/opt/trn_rl_repo")

import ml_dtypes
import numpy as np

import concourse.bacc as bacc
import concourse.mybir as mybir
import concourse.tile as tile
from concourse.bass_utils import run_bass_kernel_spmd

B = 128        # batch
R = 8192       # rows of sparse matrix / output features
C = 16384      # cols of sparse matrix / input features
NCORES = 8
RC = R // NCORES       # rows (output features) per core
NCH = C // 128         # contraction chunks of 128
NT = RC // 512         # 512-wide PSUM column tiles per core (= 2)

E3M4_MAX = 15.5        # largest finite float8_e3m4

# W DMA tile sizes in c-chunks: a small leading tile cuts time-to-first-
# matmul, but too many small tiles serialize on HWDGE descriptor-gen
# (~0.63us per dma_start), starving the PE early.  The first N_HEAD chunks
# load unsplit (full 1024 rows) from a separate head tensor so the critical
# first matmuls wait on a single DMA+semaphore per group.
W_HEAD_GROUPS = [2, 2]
N_HEAD = sum(W_HEAD_GROUPS)
W_GROUPS = [4] + [8] * 15     # split lo/hi across the two rings
# x^T DMA slice sizes in c-chunks, same idea
X_SLICES = [4, 12] + [8] * 14
# Dummy matmuls ramp the PE p-state AND delay the first real matmul until
# the DMA pipeline has built a W cushion (see module docstring).
N_WARMUP = 20


def _starts(sizes, base=0, total=NCH):
    out, acc = [], base
    for s in sizes:
        out.append(acc)
        acc += s
    assert acc == total
    return out


W_HEAD_START = _starts(W_HEAD_GROUPS, 0, N_HEAD)
W_START = _starts(W_GROUPS, N_HEAD)
X_START = _starts(X_SLICES)


def _schedule():
    """DMA issue schedule: events in PE-need order, greedily assigned to the
    ring with fewer cumulative bytes.  Head W groups load unsplit; later W
    groups are split into lo/hi row halves (one per event) so both rings
    carry the same W load."""
    ev = []  # (need_chunk, prio, kind, idx, bytes)
    for s, st in enumerate(X_START):
        ev.append((st, 0, "x", s, X_SLICES[s] * B * 2 * 128))
    for g, st in enumerate(W_HEAD_START):
        ev.append((st, 1, "wh", g, W_HEAD_GROUPS[g] * RC * 128))
    for g, st in enumerate(W_START):
        ev.append((st, 1, "wlo", g, W_GROUPS[g] * 512 * 128))
        ev.append((st, 2, "whi", g, W_GROUPS[g] * 512 * 128))
    ev.sort()
    load = [0, 0]
    out = []  # (kind, idx, ring)
    for _, _, kind, idx, nb in ev:
        r = 0 if load[0] <= load[1] else 1
        load[r] += nb
        out.append((kind, idx, r))
    return out


def _densify_tiled(vals, rows, cols):
    """w_t[p, ch, r] = sum of vals at (row=r, col=ch*128+p): dense M^T
    pre-tiled for the SBUF partition layout, [128, NCH, R] f32."""
    w_t = np.zeros((128, NCH, R), dtype=np.float32)
    np.add.at(w_t, (cols % 128, cols // 128, rows), vals)
    return w_t


def _build_nc():
    nc = bacc.Bacc("TRN2", target_bir_lowering=False, debug=False)
    # x^T pre-tiled on host: xt[p, ch, b] = x[b, ch*128+p] / s
    xt_d = nc.dram_tensor(
        "xt", [128, NCH * B], mybir.dt.float16, kind="ExternalInput"
    )
    # W pre-tiled on host.  whead: first N_HEAD chunks, full row width.
    # w{lo,hi}[p, ch, r] = s * W[ch*128+p, core_rows[half*512 + r]]
    whead_d = nc.dram_tensor(
        "whead", [128, N_HEAD, RC], mybir.dt.float8e3, kind="ExternalInput"
    )
    wlo_d = nc.dram_tensor(
        "wlo", [128, NCH, 512], mybir.dt.float8e3, kind="ExternalInput"
    )
    whi_d = nc.dram_tensor(
        "whi", [128, NCH, 512], mybir.dt.float8e3, kind="ExternalInput"
    )
    y_d = nc.dram_tensor(
        "y", [128, RC], mybir.dt.bfloat16, kind="ExternalOutput"
    )
    w_d = [wlo_d, whi_d]
    rings = [nc.sync, nc.scalar]

    with tile.TileContext(nc) as tc:
        with (
            tc.tile_pool(name="xsb", bufs=1) as xpool,
            tc.tile_pool(name="whead", bufs=len(W_HEAD_GROUPS)) as whpool,
            tc.tile_pool(name="wlosb", bufs=16) as wlopool,
            tc.tile_pool(name="whisb", bufs=16) as whipool,
            tc.tile_pool(name="warm", bufs=1) as mpool,
            tc.tile_pool(name="ysb", bufs=1) as ypool,
            tc.tile_pool(name="ps", bufs=NT, space="PSUM") as ppool,
            tc.tile_pool(name="pwarm", bufs=1, space="PSUM") as wppool,
        ):
            # --- PE warmup: ramp the p-state while the first DMAs land ---
            warm = mpool.tile([128, 512], mybir.dt.float16)
            nc.gpsimd.memset(warm[:], 0.0)
            wpsum = wppool.tile([128, 256], mybir.dt.float32)
            for _ in range(N_WARMUP):
                nc.tensor.matmul(
                    wpsum[:], warm[:, :128], warm[:, :256], start=True, stop=True
                )

            x_t = xpool.tile([128, NCH, B], mybir.dt.float16)
            wpools = [wlopool, whipool]
            w_tiles = [[], []]
            wh_tiles = []
            # issue every DMA in need order; tile-pool reuse semaphores pace
            # the W stream ~16 groups ahead of the matmuls
            for kind, idx, ring in _schedule():
                if kind == "x":
                    st, ln = X_START[idx], X_SLICES[idx]
                    rings[ring].dma_start(
                        out=x_t[:, st:st + ln, :],
                        in_=xt_d[:, st * B:(st + ln) * B],
                    )
                elif kind == "wh":
                    st, ln = W_HEAD_START[idx], W_HEAD_GROUPS[idx]
                    w_t = whpool.tile([128, ln, RC], mybir.dt.float8e3)
                    wh_tiles.append(w_t)
                    rings[ring].dma_start(
                        out=w_t[:], in_=whead_d[:, st:st + ln, :]
                    )
                else:
                    t = 0 if kind == "wlo" else 1
                    st, ln = W_START[idx], W_GROUPS[idx]
                    w_t = wpools[t].tile([128, ln, 512], mybir.dt.float8e3)
                    w_tiles[t].append(w_t)
                    rings[ring].dma_start(
                        out=w_t[:], in_=w_d[t][:, st:st + ln, :]
                    )
            psums = [
                ppool.tile(
                    [128, 512], mybir.dt.float32, name=f"psum{t}", tag=f"psum{t}"
                )
                for t in range(NT)
            ]
            for g, (wst, wln) in enumerate(zip(W_HEAD_START, W_HEAD_GROUPS)):
                for i in range(wln):
                    ch = wst + i
                    for t in range(NT):
                        nc.tensor.matmul(
                            psums[t][:],
                            x_t[:, ch, :],
                            wh_tiles[g][:, i, t * 512:(t + 1) * 512],
                            start=(ch == 0),
                            stop=False,
                        )
            last_g = len(W_GROUPS) - 1
            for g, (wst, wln) in enumerate(zip(W_START, W_GROUPS)):
                if g < last_g:
                    order = [(i, t) for i in range(wln) for t in range(NT)]
                else:
                    # final group: finish psum0 first so its cast+store
                    # overlap the remaining psum1 matmuls
                    order = [(i, t) for t in range(NT) for i in range(wln)]
                for i, t in order:
                    ch = wst + i
                    nc.tensor.matmul(
                        psums[t][:],
                        x_t[:, ch, :],
                        w_tiles[t][g][:, i, :],
                        start=False,
                        stop=(ch == NCH - 1),
                    )
            # tail: cast each 512-row half to bf16 via DVE (halves the copy
            # and store; adds ~0.2% rounding, negligible next to the 1.3%
            # W-quantization error), store on its own ring.  The last half
            # (psum1, the final stop) casts in two 256-col pieces so its
            # first store overlaps the second cast.
            y_t = ypool.tile([128, RC], mybir.dt.bfloat16)
            nc.vector.tensor_copy(out=y_t[:, 0:512], in_=psums[0][:])
            rings[0].dma_start(out=y_d[:, 0:512], in_=y_t[:, 0:512])
            for h in range(2):
                lo, hi = 512 + h * 256, 512 + (h + 1) * 256
                nc.vector.tensor_copy(
                    out=y_t[:, lo:hi], in_=psums[1][:, h * 256:(h + 1) * 256]
                )
                rings[1].dma_start(out=y_d[:, lo:hi], in_=y_t[:, lo:hi])
    nc.compile()
    return nc


_CACHE = {}
_TRACE = False  # set by bench harness to capture an NTFF profile


def _get_nc():
    if "nc" not in _CACHE:
        _CACHE["nc"] = _build_nc()
    return _CACHE["nc"]


def kernel(x_batched, M_vals, M_row_idx, M_col_idx, _want_results=False, **_):
    x = np.asarray(x_batched, dtype=np.float32)
    vals = np.asarray(M_vals, dtype=np.float32)
    rows = np.asarray(M_row_idx, dtype=np.int64)
    cols = np.asarray(M_col_idx, dtype=np.int64)

    w_t = _densify_tiled(vals, rows, cols)               # [128, NCH, R] f32
    s = E3M4_MAX / float(np.abs(w_t).max()) * 0.999
    w_t8 = (w_t * s).astype(ml_dtypes.float8_e3m4)
    xt = np.ascontiguousarray(
        (x.T / s).reshape(NCH, 128, B).transpose(1, 0, 2).reshape(128, NCH * B)
    ).astype(np.float16)

    nc = _get_nc()
    in_maps = [
        {
            "xt": xt,
            "whead": np.ascontiguousarray(
                w_t8[:, :N_HEAD, m * RC:(m + 1) * RC]
            ),
            "wlo": np.ascontiguousarray(
                w_t8[:, :, m * RC:m * RC + 512]
            ),
            "whi": np.ascontiguousarray(
                w_t8[:, :, m * RC + 512:(m + 1) * RC]
            ),
        }
        for m in range(NCORES)
    ]
    res = run_bass_kernel_spmd(
        nc, in_maps, core_ids=list(range(NCORES)), trace=_TRACE
    )

    y = np.empty((B, R), dtype=np.float32)
    for m in range(NCORES):
        y[:, m * RC:(m + 1) * RC] = np.asarray(
            res.results[m]["y"]
        ).astype(np.float32)
    if _want_results:
        return y, res
    return y


# revision 70
# speedup vs baseline: 1.0176x; 1.0036x over previous
"""Batched sparse-dense matmul (COO SpMM) on 8 Trainium2 NeuronCores.

Problem: y[b, r] = sum_k vals[k] * x[b, cols[k]] where rows[k] == r.
  x: [128, 16384] f32, vals/rows/cols: [524288], y: [128, 8192] f32.

Strategy: at 0.39% density with a full 128-wide batch, a dense matmul
y = x @ M^T beats any per-nonzero gather on this hardware (SWDGE
descriptor generation costs ~4-9ns per gathered element, and GPSIMD
scatter/gather ops run at ~1 elem/cycle-per-partition — both orders of
magnitude off the tensor engine).  So:
  - Host: densify M^T into W [C, R], shard W's output columns across
    the 8 cores (1024 rows each), and pre-tile both x^T and W for the
    SBUF partition layout.
  - Dtypes: W is cast to fp8e3 (e3m4, 4-bit mantissa) after scaling so
    max|W| lands at the e3m4 max (15.5); the inverse scale is folded
    into x, which stays fp16.  Measured rel error ~1.34e-2 (the e4m3
    variant fails the 2e-2 gate at 2.6e-2; fp16 W gives 3e-4 but
    doubles the dominant DMA stream).  W traffic halves vs fp16:
    16 MiB/core, leaving the PE moving-column stream (131072 cols
    @ 2.4 GHz = 55us) as the critical path.
  - Device (per core): keep x^T resident in SBUF as 128 [128c x 128b]
    fp16 chunks (the matmul's stationary operand); stream W from HBM
    split into row-halves, one per HWDGE ring (sync/scalar) so both
    rings carry identical load; x slices are injected into whichever
    ring is less loaded, in PE-need order; the first chunks load
    unsplit from a small head tensor so the critical first matmuls
    wait on one DMA+semaphore.  PSUM accumulates over the 128 c-chunks
    into y[128b x 1024r] fp32; DVE casts to bf16 and both rings store.
  - Latency trims (per trace): a burst of dummy matmuls on scratch SBUF
    ramps the PE p-state (0.65 -> 2.4 GHz needs ~3us busy) and delays
    the first real matmul to ~12.5us, by when the DMA pipeline (which
    also ramps, ~250 GB/s over its first 10us) has buffered enough W
    that the stream runs gap-free; deep W prefetch (16 tiles/ring)
    rides out HBM slow episodes; the final W group runs all psum0
    matmuls before psum1's so psum0's cast+store hide under the
    stream's last ~1.7us.
  - Host: concatenate the per-core row slices (upcast bf16 -> f32).

Known dead ends (measured): fp8e4 DoubleRow halves PE cycles but fails
the accuracy gate full-width (2.6e-2) and dies with a redacted runtime
INVALID_ARGUMENT when mixed in for the last 12 chunks at full kernel
scale (works in isolation); int8 is not a tensor-engine dtype; GPSIMD
scatter_add/ap_gather and SWDGE per-nnz gathers are 1-2 orders of
magnitude too slow for the 67M-MAC sparse form.
"""

import sys

sys.path.insert(0, "/opt/trn_rl_repo")

import ml_dtypes
import numpy as np

import concourse.bacc as bacc
import concourse.mybir as mybir
import concourse.tile as tile
from concourse.bass_utils import run_bass_kernel_spmd

B = 128        # batch
R = 8192       # rows of sparse matrix / output features
C = 16384      # cols of sparse matrix / input features
NCORES = 8
RC = R // NCORES       # rows (output features) per core
NCH = C // 128         # contraction chunks of 128
NT = RC // 512         # 512-wide PSUM column tiles per core (= 2)

E3M4_MAX = 15.5        # largest finite float8_e3m4

# W DMA tile sizes in c-chunks: a small leading tile cuts time-to-first-
# matmul, but too many small tiles serialize on HWDGE descriptor-gen
# (~0.63us per dma_start), starving the PE early.  The first N_HEAD chunks
# load unsplit (full 1024 rows) from a separate head tensor so the critical
# first matmuls wait on a single DMA+semaphore per group.
W_HEAD_GROUPS = [2, 2]
N_HEAD = sum(W_HEAD_GROUPS)
W_GROUPS = [4] + [8] * 15     # split lo/hi across the two rings
# x^T DMA slice sizes in c-chunks, same idea
X_SLICES = [4, 12] + [8] * 14
# Dummy matmuls ramp the PE p-state AND delay the first real matmul until
# the DMA pipeline has built a W cushion (see module docstring).
N_WARMUP = 24


def _starts(sizes, base=0, total=NCH):
    out, acc = [], base
    for s in sizes:
        out.append(acc)
        acc += s
    assert acc == total
    return out


W_HEAD_START = _starts(W_HEAD_GROUPS, 0, N_HEAD)
W_START = _starts(W_GROUPS, N_HEAD)
X_START = _starts(X_SLICES)


def _schedule():
    """DMA issue schedule: events in PE-need order, greedily assigned to the
    ring with fewer cumulative bytes.  Head W groups load unsplit; later W
    groups are split into lo/hi row halves (one per event) so both rings
    carry the same W load."""
    ev = []  # (need_chunk, prio, kind, idx, bytes)
    for s, st in enumerate(X_START):
        ev.append((st, 0, "x", s, X_SLICES[s] * B * 2 * 128))
    for g, st in enumerate(W_HEAD_START):
        ev.append((st, 1, "wh", g, W_HEAD_GROUPS[g] * RC * 128))
    for g, st in enumerate(W_START):
        ev.append((st, 1, "wlo", g, W_GROUPS[g] * 512 * 128))
        ev.append((st, 2, "whi", g, W_GROUPS[g] * 512 * 128))
    ev.sort()
    load = [0, 0]
    out = []  # (kind, idx, ring)
    for _, _, kind, idx, nb in ev:
        r = 0 if load[0] <= load[1] else 1
        load[r] += nb
        out.append((kind, idx, r))
    return out


def _densify_tiled(vals, rows, cols):
    """w_t[p, ch, r] = sum of vals at (row=r, col=ch*128+p): dense M^T
    pre-tiled for the SBUF partition layout, [128, NCH, R] f32."""
    w_t = np.zeros((128, NCH, R), dtype=np.float32)
    np.add.at(w_t, (cols % 128, cols // 128, rows), vals)
    return w_t


def _build_nc():
    nc = bacc.Bacc("TRN2", target_bir_lowering=False, debug=False)
    # x^T pre-tiled on host: xt[p, ch, b] = x[b, ch*128+p] / s
    xt_d = nc.dram_tensor(
        "xt", [128, NCH * B], mybir.dt.float16, kind="ExternalInput"
    )
    # W pre-tiled on host.  whead: first N_HEAD chunks, full row width.
    # w{lo,hi}[p, ch, r] = s * W[ch*128+p, core_rows[half*512 + r]]
    whead_d = nc.dram_tensor(
        "whead", [128, N_HEAD, RC], mybir.dt.float8e3, kind="ExternalInput"
    )
    wlo_d = nc.dram_tensor(
        "wlo", [128, NCH, 512], mybir.dt.float8e3, kind="ExternalInput"
    )
    whi_d = nc.dram_tensor(
        "whi", [128, NCH, 512], mybir.dt.float8e3, kind="ExternalInput"
    )
    y_d = nc.dram_tensor(
        "y", [128, RC], mybir.dt.bfloat16, kind="ExternalOutput"
    )
    w_d = [wlo_d, whi_d]
    rings = [nc.sync, nc.scalar]

    with tile.TileContext(nc) as tc:
        with (
            tc.tile_pool(name="xsb", bufs=1) as xpool,
            tc.tile_pool(name="whead", bufs=len(W_HEAD_GROUPS)) as whpool,
            tc.tile_pool(name="wlosb", bufs=16) as wlopool,
            tc.tile_pool(name="whisb", bufs=16) as whipool,
            tc.tile_pool(name="warm", bufs=1) as mpool,
            tc.tile_pool(name="ysb", bufs=1) as ypool,
            tc.tile_pool(name="ps", bufs=NT, space="PSUM") as ppool,
            tc.tile_pool(name="pwarm", bufs=1, space="PSUM") as wppool,
        ):
            # --- PE warmup: ramp the p-state while the first DMAs land ---
            warm = mpool.tile([128, 512], mybir.dt.float16)
            nc.gpsimd.memset(warm[:], 0.0)
            wpsum = wppool.tile([128, 256], mybir.dt.float32)
            for _ in range(N_WARMUP):
                nc.tensor.matmul(
                    wpsum[:], warm[:, :128], warm[:, :256], start=True, stop=True
                )

            x_t = xpool.tile([128, NCH, B], mybir.dt.float16)
            wpools = [wlopool, whipool]
            w_tiles = [[], []]
            wh_tiles = []
            # issue every DMA in need order; tile-pool reuse semaphores pace
            # the W stream ~16 groups ahead of the matmuls
            for kind, idx, ring in _schedule():
                if kind == "x":
                    st, ln = X_START[idx], X_SLICES[idx]
                    rings[ring].dma_start(
                        out=x_t[:, st:st + ln, :],
                        in_=xt_d[:, st * B:(st + ln) * B],
                    )
                elif kind == "wh":
                    st, ln = W_HEAD_START[idx], W_HEAD_GROUPS[idx]
                    w_t = whpool.tile([128, ln, RC], mybir.dt.float8e3)
                    wh_tiles.append(w_t)
                    rings[ring].dma_start(
                        out=w_t[:], in_=whead_d[:, st:st + ln, :]
                    )
                else:
                    t = 0 if kind == "wlo" else 1
                    st, ln = W_START[idx], W_GROUPS[idx]
                    w_t = wpools[t].tile([128, ln, 512], mybir.dt.float8e3)
                    w_tiles[t].append(w_t)
                    rings[ring].dma_start(
                        out=w_t[:], in_=w_d[t][:, st:st + ln, :]
                    )
            psums = [
                ppool.tile(
                    [128, 512], mybir.dt.float32, name=f"psum{t}", tag=f"psum{t}"
                )
                for t in range(NT)
            ]
            for g, (wst, wln) in enumerate(zip(W_HEAD_START, W_HEAD_GROUPS)):
                for i in range(wln):
                    ch = wst + i
                    for t in range(NT):
                        nc.tensor.matmul(
                            psums[t][:],
                            x_t[:, ch, :],
                            wh_tiles[g][:, i, t * 512:(t + 1) * 512],
                            start=(ch == 0),
                            stop=False,
                        )
            last_g = len(W_GROUPS) - 1
            for g, (wst, wln) in enumerate(zip(W_START, W_GROUPS)):
                if g < last_g:
                    order = [(i, t) for i in range(wln) for t in range(NT)]
                else:
                    # final group: finish psum0 first so its cast+store
                    # overlap the remaining psum1 matmuls
                    order = [(i, t) for t in range(NT) for i in range(wln)]
                for i, t in order:
                    ch = wst + i
                    nc.tensor.matmul(
                        psums[t][:],
                        x_t[:, ch, :],
                        w_tiles[t][g][:, i, :],
                        start=False,
                        stop=(ch == NCH - 1),
                    )
            # tail: cast each 512-row half to bf16 via DVE (halves the copy
            # and store; adds ~0.2% rounding, negligible next to the 1.3%
            # W-quantization error), store on its own ring.  The last half
            # (psum1, the final stop) casts in two 256-col pieces so its
            # first store overlaps the second cast.
            y_t = ypool.tile([128, RC], mybir.dt.bfloat16)
            nc.vector.tensor_copy(out=y_t[:, 0:512], in_=psums[0][:])
            rings[0].dma_start(out=y_d[:, 0:512], in_=y_t[:, 0:512])
            for h in range(2):
                lo, hi = 512 + h * 256, 512 + (h + 1) * 256
                nc.vector.tensor_copy(
                    out=y_t[:, lo:hi], in_=psums[1][:, h * 256:(h + 1) * 256]
                )
                rings[1].dma_start(out=y_d[:, lo:hi], in_=y_t[:, lo:hi])
    nc.compile()
    return nc


_CACHE = {}
_TRACE = False  # set by bench harness to capture an NTFF profile


def _get_nc():
    if "nc" not in _CACHE:
        _CACHE["nc"] = _build_nc()
    return _CACHE["nc"]


def kernel(x_batched, M_vals, M_row_idx, M_col_idx, _want_results=False, **_):
    x = np.asarray(x_batched, dtype=np.float32)
    vals = np.asarray(M_vals, dtype=np.float32)
    rows = np.asarray(M_row_idx, dtype=np.int64)
    cols = np.asarray(M_col_idx, dtype=np.int64)

    w_t = _densify_tiled(vals, rows, cols)               # [128, NCH, R] f32
    s = E3M4_MAX / float(np.abs(w_t).max()) * 0.999
    w_t8 = (w_t * s).astype(ml_dtypes.float8_e3m4)
    xt = np.ascontiguousarray(
        (x.T / s).reshape(NCH, 128, B).transpose(1, 0, 2).reshape(128, NCH * B)
    ).astype(np.float16)

    nc = _get_nc()
    in_maps = [
        {
            "xt": xt,
            "whead": np.ascontiguousarray(
                w_t8[:, :N_HEAD, m * RC:(m + 1) * RC]
            ),
            "wlo": np.ascontiguousarray(
                w_t8[:, :, m * RC:m * RC + 512]
            ),
            "whi": np.ascontiguousarray(
                w_t8[:, :, m * RC + 512:(m + 1) * RC]
            ),
        }
        for m in range(NCORES)
    ]
    res = run_bass_kernel_spmd(
        nc, in_maps, core_ids=list(range(NCORES)), trace=_TRACE
    )

    y = np.empty((B, R), dtype=np.float32)
    for m in range(NCORES):
        y[:, m * RC:(m + 1) * RC] = np.asarray(
            res.results[m]["y"]
        ).astype(np.float32)
    if _want_results:
        return y, res
    return y
